# revision 10
# baseline (speedup 1.0000x reference)
"""Trainium2 Bass kernel for the differentiable gaussian-splat renderer.

Full-input contract: kernel(**inputs) takes the unsharded inputs and returns
the full [2*16, 3, 32, 32] output.

Math (per pose):
    cam = positions @ R.T + t ;  pj = (fx*cam_x/cam_z + cx, fy*cam_y/cam_z + cy)
    w[n, p] = op_n * exp(-0.5*((px-ax_n)^2 + (py-ay_n)^2)/s_n^2)
    img = (w.T @ colors) / (w.T @ 1 + 1e-8)

The gaussian weight is separable: w = op * wx[n,px] * wy[n,py].

Sharding: 8 cores = 2 poses x 4 gaussian shards (1024 gaussians each). Each
core evaluates the FULL 128x128 image partial accumulators (num, den) for its
shard; the host sums the shard partials per pose (the all-reduce step) and
normalizes. Projection, quadratic-coefficient construction and the 3-piece
bf16 splits are all done on the host (cheap O(N) numpy); the device runs:
4 arg matmuls -> 4 exps -> X build -> 8 accumulation matmuls -> DMA out,
pipelined in chunk halves with per-half tiles so dependencies stay tight.

The exp argument g*(q-ax)^2 (q centered at 64) is a matmul of per-gaussian
quadratic coefficients [g, -2*g*ax, g*ax^2] (each split into 3 bf16 pieces,
exact to ~24 bits) against a block-diagonal pixel basis [q^2(hi,lo), q, 1].
px and py share one basis; only a 4-chunk [64, 512] block-diag basis is sent
(both coefT halves are packed at partition base 0 so they can share it).
colors*opacity is broadcast over px on-device by log-doubling copies (the
DVE runs bf16 unit-stride copies in 2x mode; a 1 MB pre-broadcast DMA or
stride-0 broadcast reads are both far slower). Everything device-side is
bf16 (tolerance 2e-2 leaves ample margin); accumulation is f32 in PSUM.
"""

import numpy as np

H = 128
W = 128
FX = 120.0
FY = 120.0
N = 4096
NPOSE = 2
NSHARD = 4           # gaussian shards (cores per pose)
NG = N // NSHARD     # 1024 gaussians per core
NCHUNK = NG // 128   # 8 chunks of 128 gaussians
F32 = np.float32

_CACHE = {}


def _quat2mat(q):
    q = np.asarray(q, dtype=np.float64)
    q = q / np.linalg.norm(q)
    w, x, y, z = q
    return np.array([
        [1 - 2 * (y * y + z * z), 2 * (x * y - z * w), 2 * (x * z + y * w)],
        [2 * (x * y + z * w), 1 - 2 * (x * x + z * z), 2 * (y * z - x * w)],
        [2 * (x * z - y * w), 2 * (y * z + x * w), 1 - 2 * (x * x + y * y)],
    ])


def _build_program():
    """Build the SPMD Bass/Tile program (same program on every core)."""
    import concourse.bacc as bacc
    import concourse.tile as tile
    import concourse.mybir as mybir
    from contextlib import ExitStack

    dt = mybir.dt.float32
    bf = mybir.dt.bfloat16
    nc = bacc.Bacc()

    # ---- DRAM I/O (per-core shapes) ----
    # ca[m, 4j+c] = channel c of gaussian (chunk j, row m), * opacity
    ca_d = nc.dram_tensor("ca", [128, 32], bf, kind="ExternalInput").ap()
    # cb cols: 0:128 coefT_x chunks0-3 | 128:256 chunks4-7 | 256:384 coefT_y
    # chunks0-3 | 384:512 chunks4-7 | 512:1024 block-diag basis (4 chunks)
    cb_d = nc.dram_tensor("cb", [64, 1024], bf, kind="ExternalInput").ap()
    out0_d = nc.dram_tensor("out0", [128, 512], bf, kind="ExternalOutput").ap()
    out1_d = nc.dram_tensor("out1", [128, 512], bf, kind="ExternalOutput").ap()

    mult = mybir.AluOpType.mult
    EXP = mybir.ActivationFunctionType.Exp

    with tile.TileContext(nc) as tc, ExitStack() as ctx:
        const = ctx.enter_context(tc.tile_pool(name="const", bufs=1))
        pools = {}
        for tag in ("px0", "px1", "py0", "py1", "po0", "po1"):
            pools[tag] = ctx.enter_context(
                tc.tile_pool(name="p" + tag, bufs=1, space="PSUM"))

        pax = [pools["px0"].tile([128, 512], dt, tag="px0", name="px0"),
               pools["px1"].tile([128, 512], dt, tag="px1", name="px1")]
        pay = [pools["py0"].tile([128, 512], dt, tag="py0", name="py0"),
               pools["py1"].tile([128, 512], dt, tag="py1", name="py1")]
        po = [pools["po0"].tile([128, 512], dt, tag="po0", name="po0"),
              pools["po1"].tile([128, 512], dt, tag="po1", name="po1")]

        cat = const.tile([128, 32], bf, tag="cat")
        nc.sync.dma_start(out=cat[:], in_=ca_d)
        cbt = const.tile([64, 1024], bf, tag="cbt")
        nc.sync.dma_start(out=cbt[:], in_=cb_d)

        coefT = {("x", 0): cbt[:, 0:128], ("x", 1): cbt[:, 128:256],
                 ("y", 0): cbt[:, 256:384], ("y", 1): cbt[:, 384:512]}
        bas = cbt[:, 512:1024]

        # ---- caw [128, (j,c,px)=4096] built by log-doubling on DVE ----
        caw = const.tile([128, 4096], bf, tag="caw")
        caw_w = caw[:].rearrange("p (w x) -> p w x", w=32)
        nc.vector.tensor_copy(out=caw_w[:, :, 0:1], in_=cat[:].unsqueeze(2))
        k = 1
        while k < 128:
            nc.vector.tensor_copy(out=caw_w[:, :, k:2 * k],
                                  in_=caw_w[:, :, 0:k])
            k *= 2
        caw_v = caw[:].rearrange("p (j c x) -> p j c x", j=8, c=4)

        # ---- arg matmuls + exp, per half ----
        wx = [const.tile([128, 512], bf, tag=f"wx{h}", name=f"wx{h}") for h in range(2)]
        wy = [const.tile([128, 512], bf, tag=f"wy{h}", name=f"wy{h}") for h in range(2)]
        for h in range(2):
            nc.tensor.matmul(pax[h][:], lhsT=coefT[("x", h)], rhs=bas,
                             start=True, stop=True)
        for h in range(2):
            nc.tensor.matmul(pay[h][:], lhsT=coefT[("y", h)], rhs=bas,
                             start=True, stop=True)
        # scalar queue order: wx halves first so X build can start early
        for h in range(2):
            nc.scalar.activation(out=wx[h][:], in_=pax[h][:], func=EXP)
        for h in range(2):
            nc.scalar.activation(out=wy[h][:], in_=pay[h][:], func=EXP)

        # ---- X_h [128, (jl,c,px)=2048] = caw * wx_h ; then 4 main matmuls
        #      po_h[py, (c,px)] += wy_j.T @ X_j per half ----
        ob = [const.tile([128, 512], bf, tag=f"ob{h}", name=f"ob{h}") for h in range(2)]
        for h in range(2):
            X = const.tile([128, 2048], bf, tag=f"X{h}")
            X_v = X[:].rearrange("p (j c x) -> p j c x", j=4, c=4)
            wx_v = wx[h][:].rearrange("p (j x) -> p j x", j=4)
            for c in range(4):
                eng = nc.gpsimd if c == 3 else nc.vector
                eng.tensor_tensor(out=X_v[:, :, c, :], in0=wx_v[:],
                                  in1=caw_v[:, 4 * h:4 * h + 4, c, :], op=mult)
            for j in range(4):
                nc.tensor.matmul(po[h][:], lhsT=wy[h][:, 128 * j:128 * j + 128],
                                 rhs=X[:, 512 * j:512 * j + 512],
                                 start=(j == 0), stop=(j == 3))
            # drain this half's partials while the other half computes
            if h == 0:
                nc.vector.tensor_copy(out=ob[0][:], in_=po[0][:])
                nc.sync.dma_start(out=out0_d, in_=ob[0][:])
        nc.scalar.activation(out=ob[1][:], in_=po[1][:],
                             func=mybir.ActivationFunctionType.Copy)
        nc.sync.dma_start(out=out1_d, in_=ob[1][:])

    nc.compile()
    return nc


def _split3(v, bf):
    """Split f64 array v into 3 bf16 pieces summing to ~24-bit accuracy."""
    s1 = v.astype(bf)
    s2 = (v - s1.astype(np.float64)).astype(bf)
    s3 = (v - s1.astype(np.float64) - s2.astype(np.float64)).astype(bf)
    return s1, s2, s3


def _host_prep(positions, colors, opacities, scales, qvec, tvec):
    """Build the 8 per-core input maps (all projection/coef math on host)."""
    import ml_dtypes
    bf = ml_dtypes.bfloat16

    positions = np.asarray(positions, dtype=np.float64)
    colors = np.asarray(colors, dtype=np.float64)
    opacities = np.asarray(opacities, dtype=np.float64)
    scales = np.asarray(scales, dtype=np.float64)
    qvec = np.asarray(qvec, dtype=np.float64)
    tvec = np.asarray(tvec, dtype=np.float64)

    gneg = -0.5 / (scales[:, 0] ** 2)            # [N]
    cav = np.concatenate([colors * opacities, opacities], axis=1)  # [N,4]

    # 4-chunk block-diagonal basis [64, 512]
    q = np.arange(128.0) - 64.0
    p2 = q * q
    p2h = p2.astype(bf)
    p2l = (p2 - p2h.astype(np.float64)).astype(bf)
    bas = np.zeros((64, 512), bf)
    for j in range(4):
        r0, c0 = 16 * j, 128 * j
        for r in (0, 2, 4):
            bas[r0 + r, c0:c0 + 128] = p2h
            bas[r0 + r + 1, c0:c0 + 128] = p2l
        for r in (6, 7, 8):
            bas[r0 + r, c0:c0 + 128] = q.astype(bf)
        for r in (9, 10, 11):
            bas[r0 + r, c0:c0 + 128] = 1.0

    def coefT_half(a_c, g_c):
        """[64, 128] bf16: rows 16jl+r, cols m, for 4 chunks."""
        out = np.zeros((64, 128), bf)
        gg = g_c.reshape(4, 128)
        b = (-2.0 * g_c * a_c).reshape(4, 128)
        cc = (g_c * a_c * a_c).reshape(4, 128)
        for j in range(4):
            a1, a2, a3 = _split3(gg[j], bf)
            for r, v in ((0, a1), (1, a1), (2, a2), (3, a2), (4, a3), (5, a3)):
                out[16 * j + r, :] = v
            b1, b2, b3 = _split3(b[j], bf)
            for r, v in ((6, b1), (7, b2), (8, b3)):
                out[16 * j + r, :] = v
            c1, c2, c3 = _split3(cc[j], bf)
            for r, v in ((9, c1), (10, c2), (11, c3)):
                out[16 * j + r, :] = v
        return out

    in_maps = []
    for p in range(NPOSE):
        R = _quat2mat(qvec[p])
        t = tvec[p]
        A = np.zeros((3, 4))
        A[0, :3] = FX * R[0]
        A[0, 3] = FX * t[0]
        A[1, :3] = FY * R[1]
        A[1, 3] = FY * t[1]
        A[2, :3] = R[2]
        A[2, 3] = t[2]
        cam = positions @ A[:, :3].T + A[:, 3]   # [N,3]
        ax = cam[:, 0] / cam[:, 2]               # centered (cx=64 -> q=px-64)
        ay = cam[:, 1] / cam[:, 2]

        for s in range(NSHARD):
            sl = slice(s * NG, s * NG + NG)
            axs, ays, gs = ax[sl], ay[sl], gneg[sl]
            cb = np.zeros((64, 1024), bf)
            cb[:, 0:128] = coefT_half(axs[0:512], gs[0:512])
            cb[:, 128:256] = coefT_half(axs[512:1024], gs[512:1024])
            cb[:, 256:384] = coefT_half(ays[0:512], gs[0:512])
            cb[:, 384:512] = coefT_half(ays[512:1024], gs[512:1024])
            cb[:, 512:1024] = bas
            # ca[m, 4j+c]
            cv = cav[sl].reshape(NCHUNK, 128, 4)  # [j, m, c]
            ca = np.ascontiguousarray(
                cv.transpose(1, 0, 2).reshape(128, 32)).astype(bf)
            in_maps.append({"ca": ca, "cb": cb})
    return in_maps


def _assemble(slabs):
    """slabs: 16 x [128, 512] partials -> [NPOSE*16, 3, 32, 32] output."""
    out = []
    for p in range(NPOSE):
        acc = np.zeros((128, 512), np.float64)
        for sl in slabs[8 * p:8 * p + 8]:
            acc += sl.astype(np.float64)
        den = acc[:, 384:512] + 1e-8             # [py, px]
        img = np.empty((H, W, 3), np.float64)
        for c in range(3):
            img[:, :, c] = acc[:, 128 * c:128 * c + 128] / den
        tiles = img.reshape(H * W, 3).reshape(16, 1024, 3)
        tiles = tiles.transpose(0, 2, 1).reshape(16, 3, 32, 32)
        out.append(tiles)
    return np.concatenate(out, axis=0).astype(F32)


def kernel(positions, colors, opacities, scales, qvec, tvec, _trace=False):
    from concourse.bass_utils import run_bass_kernel_spmd

    if "nc" not in _CACHE:
        _CACHE["nc"] = _build_program()
    nc = _CACHE["nc"]

    in_maps = _host_prep(positions, colors, opacities, scales, qvec, tvec)
    res = run_bass_kernel_spmd(nc, in_maps, core_ids=list(range(8)),
                               trace=_trace)
    slabs = []
    for c in range(8):
        slabs.append(np.asarray(res.results[c]["out0"]))
        slabs.append(np.asarray(res.results[c]["out1"]))
    out = _assemble(slabs)
    if _trace:
        _CACHE["last_result"] = res
    return out


# revision 11
# speedup vs baseline: 1.1875x; 1.1875x over previous
"""Trainium2 Bass kernel for the differentiable gaussian-splat renderer.

Full-input contract: kernel(**inputs) takes the unsharded inputs and returns
the full [2*16, 3, 32, 32] output.

Math (per pose):
    cam = positions @ R.T + t ;  pj = (fx*cam_x/cam_z + cx, fy*cam_y/cam_z + cy)
    w[n, p] = op_n * exp(-0.5*((px-ax_n)^2 + (py-ay_n)^2)/s_n^2)
    img = (w.T @ colors) / (w.T @ 1 + 1e-8)

The gaussian weight is separable: w = op * wx[n,px] * wy[n,py].

Sharding: 8 cores = 2 poses x 4 gaussian shards (1024 gaussians each). Each
core evaluates the FULL 128x128 image partial accumulators (num, den) for its
shard; the host sums the shard partials per pose (the all-reduce step) and
normalizes. Projection, quadratic-coefficient construction and the 3-piece
bf16 splits are all done on the host (cheap O(N) numpy); the device runs:
4 arg matmuls -> 4 exps -> X build -> 8 accumulation matmuls -> DMA out,
pipelined in chunk halves with per-half tiles so dependencies stay tight.

The exp argument g*(q-ax)^2 (q centered at 64) is a matmul of per-gaussian
quadratic coefficients [g, -2*g*ax, g*ax^2] (each split into 3 bf16 pieces,
exact to ~24 bits) against a block-diagonal pixel basis [q^2(hi,lo), q, 1];
px and py share the basis. colors*opacity ships replicated x8 (64 KB); the
X build broadcasts it the rest of the way with a stride-0 MIDDLE AP dim --
the innermost dim stays unit-stride so the DVE keeps its 2x bf16 mode
(stride-0 innermost runs ~8x slower, measured). Everything device-side is
bf16 (tolerance 2e-2 leaves ample margin); accumulation is f32 in PSUM.
"""

import numpy as np

H = 128
W = 128
FX = 120.0
FY = 120.0
N = 4096
NPOSE = 2
NSHARD = 4           # gaussian shards (cores per pose)
NG = N // NSHARD     # 1024 gaussians per core
NCHUNK = NG // 128   # 8 chunks of 128 gaussians
F32 = np.float32

_CACHE = {}


def _quat2mat(q):
    q = np.asarray(q, dtype=np.float64)
    q = q / np.linalg.norm(q)
    w, x, y, z = q
    return np.array([
        [1 - 2 * (y * y + z * z), 2 * (x * y - z * w), 2 * (x * z + y * w)],
        [2 * (x * y + z * w), 1 - 2 * (x * x + z * z), 2 * (y * z - x * w)],
        [2 * (x * z - y * w), 2 * (y * z + x * w), 1 - 2 * (x * x + y * y)],
    ])


def _build_program():
    """Build the SPMD Bass/Tile program (same program on every core)."""
    import concourse.bacc as bacc
    import concourse.tile as tile
    import concourse.mybir as mybir
    from contextlib import ExitStack

    dt = mybir.dt.float32
    bf = mybir.dt.bfloat16
    nc = bacc.Bacc()

    # ---- DRAM I/O (per-core shapes) ----
    # inp cols: 0:128 coefT_x | 128:256 coefT_y | 256:512 caw8
    # caw8[m, (j,c,l)] = channel c of gaussian (chunk j, row m) * opacity,
    # replicated over l=0..7
    inp_d = nc.dram_tensor("inp", [128, 512], bf, kind="ExternalInput").ap()
    # shared block-diag basis: rows 16j+r active in cols 128j+q
    bas_d = nc.dram_tensor("bas", [128, 1024], bf, kind="ExternalInput").ap()
    out0_d = nc.dram_tensor("out0", [128, 512], bf, kind="ExternalOutput").ap()
    out1_d = nc.dram_tensor("out1", [128, 512], bf, kind="ExternalOutput").ap()

    mult = mybir.AluOpType.mult
    EXP = mybir.ActivationFunctionType.Exp
    CPY = mybir.ActivationFunctionType.Copy

    with tile.TileContext(nc) as tc, ExitStack() as ctx:
        const = ctx.enter_context(tc.tile_pool(name="const", bufs=1))
        pools = {}
        for tag in ("px0", "px1", "py0", "py1", "po0", "po1"):
            pools[tag] = ctx.enter_context(
                tc.tile_pool(name="p" + tag, bufs=1, space="PSUM"))
        pax = [pools["px0"].tile([128, 512], dt, tag="px0", name="px0"),
               pools["px1"].tile([128, 512], dt, tag="px1", name="px1")]
        pay = [pools["py0"].tile([128, 512], dt, tag="py0", name="py0"),
               pools["py1"].tile([128, 512], dt, tag="py1", name="py1")]
        po = [pools["po0"].tile([128, 512], dt, tag="po0", name="po0"),
              pools["po1"].tile([128, 512], dt, tag="po1", name="po1")]

        inp = const.tile([128, 512], bf, tag="inp")
        nc.sync.dma_start(out=inp[:], in_=inp_d)
        bas = const.tile([128, 1024], bf, tag="bas")
        nc.sync.dma_start(out=bas[:], in_=bas_d)

        coefT = {"x": inp[:, 0:128], "y": inp[:, 128:256]}
        # caw8 viewed [128, j, c, l(8)]
        caw_v = inp[:, 256:512].rearrange("p (j c l) -> p j c l", j=8, c=4)

        # ---- arg matmuls + exp, per half (K=128; unused chunk rows hit the
        #      zero off-blocks of the block-diagonal basis) ----
        wx = [const.tile([128, 512], bf, tag=f"wx{h}", name=f"wx{h}")
              for h in range(2)]
        wy = [const.tile([128, 512], bf, tag=f"wy{h}", name=f"wy{h}")
              for h in range(2)]
        for h in range(2):
            nc.tensor.matmul(pax[h][:], lhsT=coefT["x"],
                             rhs=bas[:, 512 * h:512 * h + 512],
                             start=True, stop=True)
            nc.tensor.matmul(pay[h][:], lhsT=coefT["y"],
                             rhs=bas[:, 512 * h:512 * h + 512],
                             start=True, stop=True)
        # scalar order: x0 then y0 (main h0 gates on wy0), then x1, y1
        nc.scalar.activation(out=wx[0][:], in_=pax[0][:], func=EXP)
        nc.scalar.activation(out=wy[0][:], in_=pay[0][:], func=EXP)
        nc.scalar.activation(out=wx[1][:], in_=pax[1][:], func=EXP)
        nc.scalar.activation(out=wy[1][:], in_=pay[1][:], func=EXP)

        # ---- X_h [128, (jl,c,px)=2048] = caw * wx_h ; 4 main matmuls per
        #      half: po_h[py, (c,px)] += wy_j.T @ X_j ----
        ob = [const.tile([128, 512], bf, tag=f"ob{h}", name=f"ob{h}")
              for h in range(2)]
        for h in range(2):
            X = const.tile([128, 2048], bf, tag=f"X{h}", name=f"X{h}")
            X_v = X[:].rearrange("p (j c r l) -> p j c r l", j=4, c=4, r=16)
            wx_v = wx[h][:].rearrange("p (j r l) -> p j r l", j=4, r=16)
            for c in range(4):
                eng = nc.gpsimd if c == 3 else nc.vector
                src = caw_v[:, 4 * h:4 * h + 4, c, :].unsqueeze(2)
                eng.tensor_tensor(out=X_v[:, :, c, :, :], in0=wx_v[:],
                                  in1=src.broadcast_to([128, 4, 16, 8]),
                                  op=mult)
            for j in range(4):
                nc.tensor.matmul(po[h][:], lhsT=wy[h][:, 128 * j:128 * j + 128],
                                 rhs=X[:, 512 * j:512 * j + 512],
                                 start=(j == 0), stop=(j == 3))
            # drain partials (bf16) while the other half computes
            nc.scalar.activation(out=ob[h][:], in_=po[h][:], func=CPY)
            nc.sync.dma_start(out=(out0_d if h == 0 else out1_d), in_=ob[h][:])

    nc.compile()
    return nc


def _split3(v, bf):
    """Split f64 array v into 3 bf16 pieces summing to ~24-bit accuracy."""
    s1 = v.astype(bf)
    s2 = (v - s1.astype(np.float64)).astype(bf)
    s3 = (v - s1.astype(np.float64) - s2.astype(np.float64)).astype(bf)
    return s1, s2, s3


def _host_prep(positions, colors, opacities, scales, qvec, tvec):
    """Build the 8 per-core input maps (all projection/coef math on host)."""
    import ml_dtypes
    bf = ml_dtypes.bfloat16

    positions = np.asarray(positions, dtype=np.float64)
    colors = np.asarray(colors, dtype=np.float64)
    opacities = np.asarray(opacities, dtype=np.float64)
    scales = np.asarray(scales, dtype=np.float64)
    qvec = np.asarray(qvec, dtype=np.float64)
    tvec = np.asarray(tvec, dtype=np.float64)

    gneg = -0.5 / (scales[:, 0] ** 2)            # [N]
    cav = np.concatenate([colors * opacities, opacities], axis=1)  # [N,4]

    # 8-chunk block-diagonal basis [128, 1024]
    q = np.arange(128.0) - 64.0
    p2 = q * q
    p2h = p2.astype(bf)
    p2l = (p2 - p2h.astype(np.float64)).astype(bf)
    bas = np.zeros((128, 1024), bf)
    for j in range(8):
        r0, c0 = 16 * j, 128 * j
        for r in (0, 2, 4):
            bas[r0 + r, c0:c0 + 128] = p2h
            bas[r0 + r + 1, c0:c0 + 128] = p2l
        for r in (6, 7, 8):
            bas[r0 + r, c0:c0 + 128] = q.astype(bf)
        for r in (9, 10, 11):
            bas[r0 + r, c0:c0 + 128] = 1.0

    def coefT(a_c, g_c):
        """[128, 128] bf16: rows 16j+r, cols m, for 8 chunks."""
        out = np.zeros((128, 128), bf)
        gg = g_c.reshape(NCHUNK, 128)
        b = (-2.0 * g_c * a_c).reshape(NCHUNK, 128)
        cc = (g_c * a_c * a_c).reshape(NCHUNK, 128)
        for j in range(NCHUNK):
            a1, a2, a3 = _split3(gg[j], bf)
            for r, v in ((0, a1), (1, a1), (2, a2), (3, a2), (4, a3), (5, a3)):
                out[16 * j + r, :] = v
            b1, b2, b3 = _split3(b[j], bf)
            for r, v in ((6, b1), (7, b2), (8, b3)):
                out[16 * j + r, :] = v
            c1, c2, c3 = _split3(cc[j], bf)
            for r, v in ((9, c1), (10, c2), (11, c3)):
                out[16 * j + r, :] = v
        return out

    in_maps = []
    for p in range(NPOSE):
        R = _quat2mat(qvec[p])
        t = tvec[p]
        A = np.zeros((3, 4))
        A[0, :3] = FX * R[0]
        A[0, 3] = FX * t[0]
        A[1, :3] = FY * R[1]
        A[1, 3] = FY * t[1]
        A[2, :3] = R[2]
        A[2, 3] = t[2]
        cam = positions @ A[:, :3].T + A[:, 3]   # [N,3]
        ax = cam[:, 0] / cam[:, 2]               # centered (cx=64 -> q=px-64)
        ay = cam[:, 1] / cam[:, 2]

        for s in range(NSHARD):
            sl = slice(s * NG, s * NG + NG)
            inp = np.zeros((128, 512), bf)
            inp[:, 0:128] = coefT(ax[sl], gneg[sl])
            inp[:, 128:256] = coefT(ay[sl], gneg[sl])
            # caw8[m, (j,c,l)]: l = 8 replicas
            cv = cav[sl].reshape(NCHUNK, 128, 4).astype(bf)   # [j, m, c]
            caw8 = np.broadcast_to(cv.transpose(1, 0, 2)[:, :, :, None],
                                   (128, NCHUNK, 4, 8)).reshape(128, 256)
            inp[:, 256:512] = caw8
            in_maps.append({"inp": inp, "bas": bas})
    return in_maps


def _assemble(slabs):
    """slabs: 16 x [128, 512] partials -> [NPOSE*16, 3, 32, 32] output."""
    out = []
    for p in range(NPOSE):
        acc = np.zeros((128, 512), np.float64)
        for sl in slabs[8 * p:8 * p + 8]:
            acc += sl.astype(np.float64)
        den = acc[:, 384:512] + 1e-8             # [py, px]
        img = np.empty((H, W, 3), np.float64)
        for c in range(3):
            img[:, :, c] = acc[:, 128 * c:128 * c + 128] / den
        tiles = img.reshape(H * W, 3).reshape(16, 1024, 3)
        tiles = tiles.transpose(0, 2, 1).reshape(16, 3, 32, 32)
        out.append(tiles)
    return np.concatenate(out, axis=0).astype(F32)


def kernel(positions, colors, opacities, scales, qvec, tvec, _trace=False):
    from concourse.bass_utils import run_bass_kernel_spmd

    if "nc" not in _CACHE:
        _CACHE["nc"] = _build_program()
    nc = _CACHE["nc"]

    in_maps = _host_prep(positions, colors, opacities, scales, qvec, tvec)
    res = run_bass_kernel_spmd(nc, in_maps, core_ids=list(range(8)),
                               trace=_trace)
    slabs = []
    for c in range(8):
        slabs.append(np.asarray(res.results[c]["out0"]))
        slabs.append(np.asarray(res.results[c]["out1"]))
    out = _assemble(slabs)
    if _trace:
        _CACHE["last_result"] = res
    return out


# revision 12
# speedup vs baseline: 1.3505x; 1.1373x over previous
"""Trainium2 Bass kernel for the differentiable gaussian-splat renderer.

Full-input contract: kernel(**inputs) takes the unsharded inputs and returns
the full [2*16, 3, 32, 32] output.

Math (per pose):
    cam = positions @ R.T + t ;  pj = (fx*cam_x/cam_z + cx, fy*cam_y/cam_z + cy)
    w[n, p] = op_n * exp(-0.5*((px-ax_n)^2 + (py-ay_n)^2)/s_n^2)
    img = (w.T @ colors) / (w.T @ 1 + 1e-8)

The gaussian weight is separable: w = op * wx[n,px] * wy[n,py].

Sharding: 8 cores = 2 poses x 4 gaussian shards (1024 gaussians each). Each
core evaluates the FULL 128x128 image partial accumulators (num, den) for its
shard; the host sums the shard partials per pose (the all-reduce step) and
normalizes. Projection, quadratic-coefficient construction and the 3-piece
bf16 splits are all done on the host (cheap O(N) numpy); the device runs:
4 arg matmuls -> 4 exps -> X build -> 8 accumulation matmuls -> DMA out,
pipelined in chunk halves with per-half tiles so dependencies stay tight.

The exp argument g*(q-ax)^2 (q centered at 64) is a matmul of per-gaussian
quadratic coefficients [g, -2*g*ax, g*ax^2] (each split into 3 bf16 pieces,
exact to ~24 bits) against a block-diagonal pixel basis [q^2(hi,lo), q, 1];
px and py share the basis, and only a 4-chunk [128,512] basis ships: the
second chunk-half's coefficients are packed at partition rows 0:64 too, so
both halves reuse it. colors*opacity ships replicated x8 (64 KB); the X
build broadcasts it the rest of the way with stride-0 MIDDLE AP dims (the
innermost dim stays unit-stride so the DVE keeps its 2x bf16 mode; stride-0
innermost measured ~8x slower). One 4-free-dim op per half covers all four
channels, keeping everything on the vector engine (cross-engine SBUF
contention measured ~3x slowdowns). Accumulation is f32 in PSUM; everything
else is bf16 (tolerance 2e-2 leaves ample margin).
"""

import numpy as np

H = 128
W = 128
FX = 120.0
FY = 120.0
N = 4096
NPOSE = 2
NSHARD = 4           # gaussian shards (cores per pose)
NG = N // NSHARD     # 1024 gaussians per core
NCHUNK = NG // 128   # 8 chunks of 128 gaussians
F32 = np.float32

_CACHE = {}


def _quat2mat(q):
    q = np.asarray(q, dtype=np.float64)
    q = q / np.linalg.norm(q)
    w, x, y, z = q
    return np.array([
        [1 - 2 * (y * y + z * z), 2 * (x * y - z * w), 2 * (x * z + y * w)],
        [2 * (x * y + z * w), 1 - 2 * (x * x + z * z), 2 * (y * z - x * w)],
        [2 * (x * z - y * w), 2 * (y * z + x * w), 1 - 2 * (x * x + y * y)],
    ])


def _build_program():
    """Build the SPMD Bass/Tile program (same program on every core)."""
    import concourse.bacc as bacc
    import concourse.tile as tile
    import concourse.mybir as mybir
    from contextlib import ExitStack

    dt = mybir.dt.float32
    bf = mybir.dt.bfloat16
    nc = bacc.Bacc()

    # ---- DRAM I/O (per-core shapes) ----
    # ba cols: 0:512 block-diag basis (4 chunks, rows 0:64) | 512:640 coefT_x
    # chunks 0-3 | 640:768 coefT_x chunks 4-7 (also at rows 0:64)
    ba_d = nc.dram_tensor("ba", [128, 768], bf, kind="ExternalInput").ap()
    # yc cols: 0:128 coefT_y chunks 0-3 | 128:256 chunks 4-7 | 256:512 caw8
    # caw8[m, (j,c,l)] = channel c of gaussian (chunk j, row m) * opacity,
    # replicated over l=0..7
    yc_d = nc.dram_tensor("yc", [128, 512], bf, kind="ExternalInput").ap()
    out0_d = nc.dram_tensor("out0", [128, 512], bf, kind="ExternalOutput").ap()
    out1_d = nc.dram_tensor("out1", [128, 512], bf, kind="ExternalOutput").ap()

    mult = mybir.AluOpType.mult
    EXP = mybir.ActivationFunctionType.Exp
    CPY = mybir.ActivationFunctionType.Copy

    with tile.TileContext(nc) as tc, ExitStack() as ctx:
        const = ctx.enter_context(tc.tile_pool(name="const", bufs=1))
        pools = {}
        for tag in ("px0", "px1", "py0", "py1", "po0", "po1"):
            pools[tag] = ctx.enter_context(
                tc.tile_pool(name="p" + tag, bufs=1, space="PSUM"))
        pax = [pools["px0"].tile([128, 512], dt, tag="px0", name="px0"),
               pools["px1"].tile([128, 512], dt, tag="px1", name="px1")]
        pay = [pools["py0"].tile([128, 512], dt, tag="py0", name="py0"),
               pools["py1"].tile([128, 512], dt, tag="py1", name="py1")]
        po = [pools["po0"].tile([128, 512], dt, tag="po0", name="po0"),
              pools["po1"].tile([128, 512], dt, tag="po1", name="po1")]

        ba = const.tile([128, 768], bf, tag="ba")
        nc.sync.dma_start(out=ba[:], in_=ba_d)
        yc = const.tile([128, 512], bf, tag="yc")
        nc.sync.dma_start(out=yc[:], in_=yc_d)

        bas = ba[:, 0:512]
        coefT = {("x", 0): ba[:, 512:640], ("x", 1): ba[:, 640:768],
                 ("y", 0): yc[:, 0:128], ("y", 1): yc[:, 128:256]}
        # caw8 viewed [128, j, c, l(8)]
        caw_v = yc[:, 256:512].rearrange("p (j c l) -> p j c l", j=8, c=4)

        # ---- arg matmuls + exp, per half ----
        wx = [const.tile([128, 512], bf, tag=f"wx{h}", name=f"wx{h}")
              for h in range(2)]
        wy = [const.tile([128, 512], bf, tag=f"wy{h}", name=f"wy{h}")
              for h in range(2)]
        nc.tensor.matmul(pax[0][:], lhsT=coefT[("x", 0)], rhs=bas,
                         start=True, stop=True)
        nc.tensor.matmul(pay[0][:], lhsT=coefT[("y", 0)], rhs=bas,
                         start=True, stop=True)
        nc.tensor.matmul(pax[1][:], lhsT=coefT[("x", 1)], rhs=bas,
                         start=True, stop=True)
        nc.tensor.matmul(pay[1][:], lhsT=coefT[("y", 1)], rhs=bas,
                         start=True, stop=True)
        # scalar order: x0 then y0 (main h0 gates on wy0), then x1, y1
        nc.scalar.activation(out=wx[0][:], in_=pax[0][:], func=EXP)
        nc.scalar.activation(out=wy[0][:], in_=pay[0][:], func=EXP)
        nc.scalar.activation(out=wx[1][:], in_=pax[1][:], func=EXP)
        nc.scalar.activation(out=wy[1][:], in_=pay[1][:], func=EXP)

        # ---- X_h [128, (jl,c,px)=2048] = caw * wx_h (one 4-free-dim vector
        #      op per half); 4 main matmuls: po_h[py,(c,px)] += wy_j.T @ X_j
        ob = [const.tile([128, 512], bf, tag=f"ob{h}", name=f"ob{h}")
              for h in range(2)]
        for h in range(2):
            X = const.tile([128, 2048], bf, tag=f"X{h}", name=f"X{h}")
            X_v = X[:].rearrange("p (j c r l) -> p j c r l", j=4, c=4, r=16)
            wx_v = wx[h][:].rearrange("p (j r l) -> p j r l", j=4, r=16)
            wx_b = wx_v.unsqueeze(2).broadcast_to([128, 4, 4, 16, 8])
            caw_b = caw_v[:, 4 * h:4 * h + 4, :, :].unsqueeze(3)
            caw_b = caw_b.broadcast_to([128, 4, 4, 16, 8])
            nc.vector.tensor_tensor(out=X_v[:], in0=wx_b, in1=caw_b, op=mult)
            for j in range(4):
                nc.tensor.matmul(po[h][:], lhsT=wy[h][:, 128 * j:128 * j + 128],
                                 rhs=X[:, 512 * j:512 * j + 512],
                                 start=(j == 0), stop=(j == 3))
            # drain partials (bf16) while the other half computes
            nc.scalar.activation(out=ob[h][:], in_=po[h][:], func=CPY)
            nc.sync.dma_start(out=(out0_d if h == 0 else out1_d), in_=ob[h][:])

    nc.compile()
    return nc


def _split3(v, bf):
    """Split f64 array v into 3 bf16 pieces summing to ~24-bit accuracy."""
    s1 = v.astype(bf)
    s2 = (v - s1.astype(np.float64)).astype(bf)
    s3 = (v - s1.astype(np.float64) - s2.astype(np.float64)).astype(bf)
    return s1, s2, s3


def _host_prep(positions, colors, opacities, scales, qvec, tvec):
    """Build the 8 per-core input maps (all projection/coef math on host)."""
    import ml_dtypes
    bf = ml_dtypes.bfloat16

    positions = np.asarray(positions, dtype=np.float64)
    colors = np.asarray(colors, dtype=np.float64)
    opacities = np.asarray(opacities, dtype=np.float64)
    scales = np.asarray(scales, dtype=np.float64)
    qvec = np.asarray(qvec, dtype=np.float64)
    tvec = np.asarray(tvec, dtype=np.float64)

    gneg = -0.5 / (scales[:, 0] ** 2)            # [N]
    cav = np.concatenate([colors * opacities, opacities], axis=1)  # [N,4]

    # 4-chunk block-diagonal basis [128, 512] (blocks in rows 0:64)
    q = np.arange(128.0) - 64.0
    p2 = q * q
    p2h = p2.astype(bf)
    p2l = (p2 - p2h.astype(np.float64)).astype(bf)
    bas = np.zeros((128, 512), bf)
    for j in range(4):
        r0, c0 = 16 * j, 128 * j
        for r in (0, 2, 4):
            bas[r0 + r, c0:c0 + 128] = p2h
            bas[r0 + r + 1, c0:c0 + 128] = p2l
        for r in (6, 7, 8):
            bas[r0 + r, c0:c0 + 128] = q.astype(bf)
        for r in (9, 10, 11):
            bas[r0 + r, c0:c0 + 128] = 1.0

    def coefT_half(a_c, g_c):
        """[128, 128] bf16: rows 16jl+r (rows 0:64), cols m, 4 chunks."""
        out = np.zeros((128, 128), bf)
        gg = g_c.reshape(4, 128)
        b = (-2.0 * g_c * a_c).reshape(4, 128)
        cc = (g_c * a_c * a_c).reshape(4, 128)
        for j in range(4):
            a1, a2, a3 = _split3(gg[j], bf)
            for r, v in ((0, a1), (1, a1), (2, a2), (3, a2), (4, a3), (5, a3)):
                out[16 * j + r, :] = v
            b1, b2, b3 = _split3(b[j], bf)
            for r, v in ((6, b1), (7, b2), (8, b3)):
                out[16 * j + r, :] = v
            c1, c2, c3 = _split3(cc[j], bf)
            for r, v in ((9, c1), (10, c2), (11, c3)):
                out[16 * j + r, :] = v
        return out

    in_maps = []
    for p in range(NPOSE):
        R = _quat2mat(qvec[p])
        t = tvec[p]
        A = np.zeros((3, 4))
        A[0, :3] = FX * R[0]
        A[0, 3] = FX * t[0]
        A[1, :3] = FY * R[1]
        A[1, 3] = FY * t[1]
        A[2, :3] = R[2]
        A[2, 3] = t[2]
        cam = positions @ A[:, :3].T + A[:, 3]   # [N,3]
        ax = cam[:, 0] / cam[:, 2]               # centered (cx=64 -> q=px-64)
        ay = cam[:, 1] / cam[:, 2]

        for s in range(NSHARD):
            sl = slice(s * NG, s * NG + NG)
            axs, ays, gs = ax[sl], ay[sl], gneg[sl]
            ba = np.zeros((128, 768), bf)
            ba[:, 0:512] = bas
            ba[:, 512:640] = coefT_half(axs[0:512], gs[0:512])
            ba[:, 640:768] = coefT_half(axs[512:1024], gs[512:1024])
            yc = np.zeros((128, 512), bf)
            yc[:, 0:128] = coefT_half(ays[0:512], gs[0:512])
            yc[:, 128:256] = coefT_half(ays[512:1024], gs[512:1024])
            # caw8[m, (j,c,l)]: l = 8 replicas
            cv = cav[sl].reshape(NCHUNK, 128, 4).astype(bf)   # [j, m, c]
            yc[:, 256:512] = np.broadcast_to(
                cv.transpose(1, 0, 2)[:, :, :, None],
                (128, NCHUNK, 4, 8)).reshape(128, 256)
            in_maps.append({"ba": ba, "yc": yc})
    return in_maps


def _assemble(slabs):
    """slabs: 16 x [128, 512] partials -> [NPOSE*16, 3, 32, 32] output."""
    out = []
    for p in range(NPOSE):
        acc = np.zeros((128, 512), np.float64)
        for sl in slabs[8 * p:8 * p + 8]:
            acc += sl.astype(np.float64)
        den = acc[:, 384:512] + 1e-8             # [py, px]
        img = np.empty((H, W, 3), np.float64)
        for c in range(3):
            img[:, :, c] = acc[:, 128 * c:128 * c + 128] / den
        tiles = img.reshape(H * W, 3).reshape(16, 1024, 3)
        tiles = tiles.transpose(0, 2, 1).reshape(16, 3, 32, 32)
        out.append(tiles)
    return np.concatenate(out, axis=0).astype(F32)


def kernel(positions, colors, opacities, scales, qvec, tvec, _trace=False):
    from concourse.bass_utils import run_bass_kernel_spmd

    if "nc" not in _CACHE:
        _CACHE["nc"] = _build_program()
    nc = _CACHE["nc"]

    in_maps = _host_prep(positions, colors, opacities, scales, qvec, tvec)
    res = run_bass_kernel_spmd(nc, in_maps, core_ids=list(range(8)),
                               trace=_trace)
    slabs = []
    for c in range(8):
        slabs.append(np.asarray(res.results[c]["out0"]))
        slabs.append(np.asarray(res.results[c]["out1"]))
    out = _assemble(slabs)
    if _trace:
        _CACHE["last_result"] = res
    return out


# revision 13
# speedup vs baseline: 1.4424x; 1.0681x over previous
"""Trainium2 Bass kernel for the differentiable gaussian-splat renderer.

Full-input contract: kernel(**inputs) takes the unsharded inputs and returns
the full [2*16, 3, 32, 32] output.

Math (per pose):
    cam = positions @ R.T + t ;  pj = (fx*cam_x/cam_z + cx, fy*cam_y/cam_z + cy)
    w[n, p] = op_n * exp(-0.5*((px-ax_n)^2 + (py-ay_n)^2)/s_n^2)
    img = (w.T @ colors) / (w.T @ 1 + 1e-8)

The gaussian weight is separable: w = op * wx[n,px] * wy[n,py].

Sharding: 8 cores = 2 poses x 4 gaussian shards (1024 gaussians each). Each
core evaluates the FULL 128x128 image partial accumulators (num, den) for its
shard; the host sums the shard partials per pose (the all-reduce step) and
normalizes. Projection, quadratic-coefficient construction and the 3-piece
bf16 splits are all done on the host (cheap O(N) numpy); the device runs:
4 arg matmuls -> 4 exps -> X build -> 8 accumulation matmuls -> DMA out,
pipelined in chunk halves with per-half tiles so dependencies stay tight.

The exp argument g*(q-ax)^2 (q centered at 64) is a matmul of per-gaussian
quadratic coefficients [g, -2*g*ax, g*ax^2] (each split into 3 bf16 pieces,
exact to ~24 bits) against a block-diagonal pixel basis [q^2(hi,lo), q, 1];
px and py share the basis, and only a 4-chunk [128,512] basis ships: the
second chunk-half's coefficients are packed at partition rows 0:64 too, so
both halves reuse it. colors*opacity ships replicated x8 (64 KB); the X
build broadcasts it the rest of the way with stride-0 MIDDLE AP dims (the
innermost dim stays unit-stride so the DVE keeps its 2x bf16 mode; stride-0
innermost measured ~8x slower). One 4-free-dim op per half covers all four
channels, keeping everything on the vector engine (cross-engine SBUF
contention measured ~3x slowdowns). Accumulation is f32 in PSUM; everything
else is bf16 (tolerance 2e-2 leaves ample margin).
"""

import numpy as np

H = 128
W = 128
FX = 120.0
FY = 120.0
N = 4096
NPOSE = 2
NSHARD = 4           # gaussian shards (cores per pose)
NG = N // NSHARD     # 1024 gaussians per core
NCHUNK = NG // 128   # 8 chunks of 128 gaussians
F32 = np.float32

_CACHE = {}


def _quat2mat(q):
    q = np.asarray(q, dtype=np.float64)
    q = q / np.linalg.norm(q)
    w, x, y, z = q
    return np.array([
        [1 - 2 * (y * y + z * z), 2 * (x * y - z * w), 2 * (x * z + y * w)],
        [2 * (x * y + z * w), 1 - 2 * (x * x + z * z), 2 * (y * z - x * w)],
        [2 * (x * z - y * w), 2 * (y * z + x * w), 1 - 2 * (x * x + y * y)],
    ])


def _build_program():
    """Build the SPMD Bass/Tile program (same program on every core)."""
    import concourse.bacc as bacc
    import concourse.tile as tile
    import concourse.mybir as mybir
    from contextlib import ExitStack

    dt = mybir.dt.float32
    bf = mybir.dt.bfloat16
    nc = bacc.Bacc()

    # ---- DRAM I/O (per-core shapes) ----
    # ba cols: 0:512 block-diag basis (4 chunks, rows 0:64) | 512:640 coefT_x
    # chunks 0-3 | 640:768 coefT_x chunks 4-7 (also at rows 0:64)
    ba_d = nc.dram_tensor("ba", [128, 768], bf, kind="ExternalInput").ap()
    # yc cols: 0:128 coefT_y chunks 0-3 | 128:256 chunks 4-7 | 256:512 caw8
    # caw8[m, (j,c,l)] = channel c of gaussian (chunk j, row m) * opacity,
    # replicated over l=0..7
    yc_d = nc.dram_tensor("yc", [128, 512], bf, kind="ExternalInput").ap()
    out_d = nc.dram_tensor("out", [128, 512], bf, kind="ExternalOutput").ap()

    mult = mybir.AluOpType.mult
    EXP = mybir.ActivationFunctionType.Exp
    CPY = mybir.ActivationFunctionType.Copy

    with tile.TileContext(nc) as tc, ExitStack() as ctx:
        const = ctx.enter_context(tc.tile_pool(name="const", bufs=1))
        pools = {}
        for tag in ("px0", "px1", "py0", "py1", "po", "pw"):
            pools[tag] = ctx.enter_context(
                tc.tile_pool(name="p" + tag, bufs=1, space="PSUM"))
        pax = [pools["px0"].tile([128, 512], dt, tag="px0", name="px0"),
               pools["px1"].tile([128, 512], dt, tag="px1", name="px1")]
        pay = [pools["py0"].tile([128, 512], dt, tag="py0", name="py0"),
               pools["py1"].tile([128, 512], dt, tag="py1", name="py1")]
        po = pools["po"].tile([128, 512], dt, tag="po", name="po")
        pw = pools["pw"].tile([128, 512], dt, tag="pw", name="pw")

        # ---- PE p-state warmup: junk matmuls on memset tiles while the
        #      input DMAs are in flight (ramps the PE clock; no data deps) ----
        wsrc = const.tile([128, 640], bf, tag="wsrc")
        nc.gpsimd.memset(wsrc[:], 0.0)
        for _ in range(5):
            nc.tensor.matmul(pw[:], lhsT=wsrc[:, 0:128], rhs=wsrc[:, 128:640],
                             start=True, stop=True)

        ba = const.tile([128, 768], bf, tag="ba")
        nc.sync.dma_start(out=ba[:], in_=ba_d)
        yc = const.tile([128, 512], bf, tag="yc")
        nc.sync.dma_start(out=yc[:], in_=yc_d)

        bas = ba[:, 0:512]
        coefT = {("x", 0): ba[:, 512:640], ("x", 1): ba[:, 640:768],
                 ("y", 0): yc[:, 0:128], ("y", 1): yc[:, 128:256]}
        # caw8 viewed [128, j, c, l(8)]
        caw_v = yc[:, 256:512].rearrange("p (j c l) -> p j c l", j=8, c=4)

        # ---- arg matmuls + exp, per half ----
        wx = [const.tile([128, 512], bf, tag=f"wx{h}", name=f"wx{h}")
              for h in range(2)]
        wy = [const.tile([128, 512], bf, tag=f"wy{h}", name=f"wy{h}")
              for h in range(2)]
        nc.tensor.matmul(pax[0][:], lhsT=coefT[("x", 0)], rhs=bas,
                         start=True, stop=True)
        nc.tensor.matmul(pay[0][:], lhsT=coefT[("y", 0)], rhs=bas,
                         start=True, stop=True)
        nc.tensor.matmul(pax[1][:], lhsT=coefT[("x", 1)], rhs=bas,
                         start=True, stop=True)
        nc.tensor.matmul(pay[1][:], lhsT=coefT[("y", 1)], rhs=bas,
                         start=True, stop=True)
        # scalar order: x0 then y0 (main h0 gates on wy0), then x1, y1
        nc.scalar.activation(out=wx[0][:], in_=pax[0][:], func=EXP)
        nc.scalar.activation(out=wy[0][:], in_=pay[0][:], func=EXP)
        nc.scalar.activation(out=wx[1][:], in_=pax[1][:], func=EXP)
        nc.scalar.activation(out=wy[1][:], in_=pay[1][:], func=EXP)

        # ---- X_h [128, (jl,c,px)=2048] = caw * wx_h (one 4-free-dim vector
        #      op per half); 4 main matmuls: po_h[py,(c,px)] += wy_j.T @ X_j
        ob = const.tile([128, 512], bf, tag="ob", name="ob")
        for h in range(2):
            X = const.tile([128, 2048], bf, tag=f"X{h}", name=f"X{h}")
            X_v = X[:].rearrange("p (j c r l) -> p j c r l", j=4, c=4, r=16)
            wx_v = wx[h][:].rearrange("p (j r l) -> p j r l", j=4, r=16)
            for pr in range(2):
                j0 = 2 * pr
                wx_b = wx_v[:, j0:j0 + 2, :, :].unsqueeze(2)
                wx_b = wx_b.broadcast_to([128, 2, 4, 16, 8])
                caw_b = caw_v[:, 4 * h + j0:4 * h + j0 + 2, :, :].unsqueeze(3)
                caw_b = caw_b.broadcast_to([128, 2, 4, 16, 8])
                nc.vector.tensor_tensor(out=X_v[:, j0:j0 + 2], in0=wx_b,
                                        in1=caw_b, op=mult)
                for j in (j0, j0 + 1):
                    nc.tensor.matmul(
                        po[:], lhsT=wy[h][:, 128 * j:128 * j + 128],
                        rhs=X[:, 512 * j:512 * j + 512],
                        start=(h == 0 and j == 0), stop=(h == 1 and j == 3))
        nc.scalar.activation(out=ob[:], in_=po[:], func=CPY)
        nc.sync.dma_start(out=out_d, in_=ob[:])

    nc.compile()
    return nc


def _split3(v, bf):
    """Split f64 array v into 3 bf16 pieces summing to ~24-bit accuracy."""
    s1 = v.astype(bf)
    s2 = (v - s1.astype(np.float64)).astype(bf)
    s3 = (v - s1.astype(np.float64) - s2.astype(np.float64)).astype(bf)
    return s1, s2, s3


def _host_prep(positions, colors, opacities, scales, qvec, tvec):
    """Build the 8 per-core input maps (all projection/coef math on host)."""
    import ml_dtypes
    bf = ml_dtypes.bfloat16

    positions = np.asarray(positions, dtype=np.float64)
    colors = np.asarray(colors, dtype=np.float64)
    opacities = np.asarray(opacities, dtype=np.float64)
    scales = np.asarray(scales, dtype=np.float64)
    qvec = np.asarray(qvec, dtype=np.float64)
    tvec = np.asarray(tvec, dtype=np.float64)

    gneg = -0.5 / (scales[:, 0] ** 2)            # [N]
    cav = np.concatenate([colors * opacities, opacities], axis=1)  # [N,4]

    # 4-chunk block-diagonal basis [128, 512] (blocks in rows 0:64)
    q = np.arange(128.0) - 64.0
    p2 = q * q
    p2h = p2.astype(bf)
    p2l = (p2 - p2h.astype(np.float64)).astype(bf)
    bas = np.zeros((128, 512), bf)
    for j in range(4):
        r0, c0 = 16 * j, 128 * j
        for r in (0, 2, 4):
            bas[r0 + r, c0:c0 + 128] = p2h
            bas[r0 + r + 1, c0:c0 + 128] = p2l
        for r in (6, 7, 8):
            bas[r0 + r, c0:c0 + 128] = q.astype(bf)
        for r in (9, 10, 11):
            bas[r0 + r, c0:c0 + 128] = 1.0

    def coefT_half(a_c, g_c):
        """[128, 128] bf16: rows 16jl+r (rows 0:64), cols m, 4 chunks."""
        out = np.zeros((128, 128), bf)
        gg = g_c.reshape(4, 128)
        b = (-2.0 * g_c * a_c).reshape(4, 128)
        cc = (g_c * a_c * a_c).reshape(4, 128)
        for j in range(4):
            a1, a2, a3 = _split3(gg[j], bf)
            for r, v in ((0, a1), (1, a1), (2, a2), (3, a2), (4, a3), (5, a3)):
                out[16 * j + r, :] = v
            b1, b2, b3 = _split3(b[j], bf)
            for r, v in ((6, b1), (7, b2), (8, b3)):
                out[16 * j + r, :] = v
            c1, c2, c3 = _split3(cc[j], bf)
            for r, v in ((9, c1), (10, c2), (11, c3)):
                out[16 * j + r, :] = v
        return out

    in_maps = []
    for p in range(NPOSE):
        R = _quat2mat(qvec[p])
        t = tvec[p]
        A = np.zeros((3, 4))
        A[0, :3] = FX * R[0]
        A[0, 3] = FX * t[0]
        A[1, :3] = FY * R[1]
        A[1, 3] = FY * t[1]
        A[2, :3] = R[2]
        A[2, 3] = t[2]
        cam = positions @ A[:, :3].T + A[:, 3]   # [N,3]
        ax = cam[:, 0] / cam[:, 2]               # centered (cx=64 -> q=px-64)
        ay = cam[:, 1] / cam[:, 2]

        for s in range(NSHARD):
            sl = slice(s * NG, s * NG + NG)
            axs, ays, gs = ax[sl], ay[sl], gneg[sl]
            ba = np.zeros((128, 768), bf)
            ba[:, 0:512] = bas
            ba[:, 512:640] = coefT_half(axs[0:512], gs[0:512])
            ba[:, 640:768] = coefT_half(axs[512:1024], gs[512:1024])
            yc = np.zeros((128, 512), bf)
            yc[:, 0:128] = coefT_half(ays[0:512], gs[0:512])
            yc[:, 128:256] = coefT_half(ays[512:1024], gs[512:1024])
            # caw8[m, (j,c,l)]: l = 8 replicas
            cv = cav[sl].reshape(NCHUNK, 128, 4).astype(bf)   # [j, m, c]
            yc[:, 256:512] = np.broadcast_to(
                cv.transpose(1, 0, 2)[:, :, :, None],
                (128, NCHUNK, 4, 8)).reshape(128, 256)
            in_maps.append({"ba": ba, "yc": yc})
    return in_maps


def _assemble(slabs):
    """slabs: 8 x [128, 512] partials -> [NPOSE*16, 3, 32, 32] output."""
    out = []
    for p in range(NPOSE):
        acc = np.zeros((128, 512), np.float64)
        for sl in slabs[4 * p:4 * p + 4]:
            acc += sl.astype(np.float64)
        den = acc[:, 384:512] + 1e-8             # [py, px]
        img = np.empty((H, W, 3), np.float64)
        for c in range(3):
            img[:, :, c] = acc[:, 128 * c:128 * c + 128] / den
        tiles = img.reshape(H * W, 3).reshape(16, 1024, 3)
        tiles = tiles.transpose(0, 2, 1).reshape(16, 3, 32, 32)
        out.append(tiles)
    return np.concatenate(out, axis=0).astype(F32)


def kernel(positions, colors, opacities, scales, qvec, tvec, _trace=False):
    from concourse.bass_utils import run_bass_kernel_spmd

    if "nc" not in _CACHE:
        _CACHE["nc"] = _build_program()
    nc = _CACHE["nc"]

    in_maps = _host_prep(positions, colors, opacities, scales, qvec, tvec)
    res = run_bass_kernel_spmd(nc, in_maps, core_ids=list(range(8)),
                               trace=_trace)
    slabs = [np.asarray(res.results[c]["out"]) for c in range(8)]
    out = _assemble(slabs)
    if _trace:
        _CACHE["last_result"] = res
    return out


# revision 14
# speedup vs baseline: 1.4742x; 1.0221x over previous
"""Trainium2 Bass kernel for the differentiable gaussian-splat renderer.

Full-input contract: kernel(**inputs) takes the unsharded inputs and returns
the full [2*16, 3, 32, 32] output.

Math (per pose):
    cam = positions @ R.T + t ;  pj = (fx*cam_x/cam_z + cx, fy*cam_y/cam_z + cy)
    w[n, p] = op_n * exp(-0.5*((px-ax_n)^2 + (py-ay_n)^2)/s_n^2)
    img = (w.T @ colors) / (w.T @ 1 + 1e-8)

The gaussian weight is separable: w = op * wx[n,px] * wy[n,py].

Sharding: 8 cores = 2 poses x 4 gaussian shards (1024 gaussians each). Each
core evaluates the FULL 128x128 image partial accumulators (num, den) for its
shard; the host sums the shard partials per pose (the all-reduce step) and
normalizes. Projection, quadratic-coefficient construction and the 3-piece
bf16 splits are all done on the host (cheap O(N) numpy); the device runs:
4 arg matmuls -> 4 exps -> X build -> 8 accumulation matmuls -> DMA out,
pipelined in chunk halves with per-half tiles so dependencies stay tight.

The exp argument g*(q-ax)^2 (q centered at 64) is a matmul of per-gaussian
quadratic coefficients [g, -2*g*ax, g*ax^2] (each split into 3 bf16 pieces,
exact to ~24 bits) against a block-diagonal pixel basis [q^2(hi,lo), q, 1];
px and py share the basis, and only a 4-chunk [128,512] basis ships: the
second chunk-half's coefficients are packed at partition rows 0:64 too, so
both halves reuse it. colors*opacity ships replicated x8 (64 KB); the X
build broadcasts it the rest of the way with stride-0 MIDDLE AP dims (the
innermost dim stays unit-stride so the DVE keeps its 2x bf16 mode; stride-0
innermost measured ~8x slower). One 4-free-dim op per half covers all four
channels, keeping everything on the vector engine (cross-engine SBUF
contention measured ~3x slowdowns). Accumulation is f32 in PSUM; everything
else is bf16 (tolerance 2e-2 leaves ample margin).
"""

import numpy as np

H = 128
W = 128
FX = 120.0
FY = 120.0
N = 4096
NPOSE = 2
NSHARD = 4           # gaussian shards (cores per pose)
NG = N // NSHARD     # 1024 gaussians per core
NCHUNK = NG // 128   # 8 chunks of 128 gaussians
F32 = np.float32

_CACHE = {}


def _quat2mat(q):
    q = np.asarray(q, dtype=np.float64)
    q = q / np.linalg.norm(q)
    w, x, y, z = q
    return np.array([
        [1 - 2 * (y * y + z * z), 2 * (x * y - z * w), 2 * (x * z + y * w)],
        [2 * (x * y + z * w), 1 - 2 * (x * x + z * z), 2 * (y * z - x * w)],
        [2 * (x * z - y * w), 2 * (y * z + x * w), 1 - 2 * (x * x + y * y)],
    ])


def _build_program():
    """Build the SPMD Bass/Tile program (same program on every core)."""
    import concourse.bacc as bacc
    import concourse.tile as tile
    import concourse.mybir as mybir
    from contextlib import ExitStack

    dt = mybir.dt.float32
    bf = mybir.dt.bfloat16
    nc = bacc.Bacc()

    # ---- DRAM I/O (per-core shapes); three DMAs sized so each lands just
    #      before its first consumer (queues run parallel, ~1.1us latency +
    #      ~180 GB/s each) ----
    # d1: 0:512 block-diag basis (4 chunks, rows 0:64) | 512:640 coefT_x h0
    d1_d = nc.dram_tensor("d1", [128, 640], bf, kind="ExternalInput").ap()
    # d2: 0:128 coefT_y h0 | 128:256 coefT_x h1
    d2_d = nc.dram_tensor("d2", [128, 256], bf, kind="ExternalInput").ap()
    # d3: 0:128 coefT_y h1 | 128:384 caw8[m,(j,c,l)] = channel c of gaussian
    # (chunk j, row m) * opacity, replicated over l=0..7
    d3_d = nc.dram_tensor("d3", [128, 384], bf, kind="ExternalInput").ap()
    out_d = nc.dram_tensor("out", [128, 512], bf, kind="ExternalOutput").ap()

    mult = mybir.AluOpType.mult
    EXP = mybir.ActivationFunctionType.Exp
    CPY = mybir.ActivationFunctionType.Copy

    with tile.TileContext(nc) as tc, ExitStack() as ctx:
        const = ctx.enter_context(tc.tile_pool(name="const", bufs=1))
        pools = {}
        for tag in ("px0", "px1", "py0", "py1", "po", "pw"):
            pools[tag] = ctx.enter_context(
                tc.tile_pool(name="p" + tag, bufs=1, space="PSUM"))
        pax = [pools["px0"].tile([128, 512], dt, tag="px0", name="px0"),
               pools["px1"].tile([128, 512], dt, tag="px1", name="px1")]
        pay = [pools["py0"].tile([128, 512], dt, tag="py0", name="py0"),
               pools["py1"].tile([128, 512], dt, tag="py1", name="py1")]
        po = pools["po"].tile([128, 512], dt, tag="po", name="po")
        pw = pools["pw"].tile([128, 512], dt, tag="pw", name="pw")

        # ---- PE p-state warmup: junk matmuls on memset tiles while the
        #      input DMAs are in flight (ramps the PE clock; no data deps) ----
        wsrc = const.tile([128, 640], bf, tag="wsrc")
        nc.gpsimd.memset(wsrc[:], 0.0)
        for _ in range(4):
            nc.tensor.matmul(pw[:], lhsT=wsrc[:, 0:128], rhs=wsrc[:, 128:640],
                             start=True, stop=True)

        d1 = const.tile([128, 640], bf, tag="d1")
        nc.sync.dma_start(out=d1[:], in_=d1_d)
        d2 = const.tile([128, 256], bf, tag="d2")
        nc.sync.dma_start(out=d2[:], in_=d2_d)
        d3 = const.tile([128, 384], bf, tag="d3")
        nc.sync.dma_start(out=d3[:], in_=d3_d)

        bas = d1[:, 0:512]
        coefT = {("x", 0): d1[:, 512:640], ("x", 1): d2[:, 128:256],
                 ("y", 0): d2[:, 0:128], ("y", 1): d3[:, 0:128]}
        # caw8 viewed [128, j, c, l(8)]
        caw_v = d3[:, 128:384].rearrange("p (j c l) -> p j c l", j=8, c=4)

        # ---- arg matmuls + exp, per half ----
        wx = [const.tile([128, 512], bf, tag=f"wx{h}", name=f"wx{h}")
              for h in range(2)]
        wy = [const.tile([128, 512], bf, tag=f"wy{h}", name=f"wy{h}")
              for h in range(2)]
        nc.tensor.matmul(pax[0][:], lhsT=coefT[("x", 0)], rhs=bas,
                         start=True, stop=True)
        nc.tensor.matmul(pay[0][:], lhsT=coefT[("y", 0)], rhs=bas,
                         start=True, stop=True)
        nc.tensor.matmul(pax[1][:], lhsT=coefT[("x", 1)], rhs=bas,
                         start=True, stop=True)
        nc.tensor.matmul(pay[1][:], lhsT=coefT[("y", 1)], rhs=bas,
                         start=True, stop=True)
        # scalar order: x0 then y0 (main h0 gates on wy0), then x1, y1
        nc.scalar.activation(out=wx[0][:], in_=pax[0][:], func=EXP)
        nc.scalar.activation(out=wy[0][:], in_=pay[0][:], func=EXP)
        nc.scalar.activation(out=wx[1][:], in_=pax[1][:], func=EXP)
        nc.scalar.activation(out=wy[1][:], in_=pay[1][:], func=EXP)

        # ---- X_h [128, (jl,c,px)=2048] = caw * wx_h (one 4-free-dim vector
        #      op per half); 4 main matmuls: po_h[py,(c,px)] += wy_j.T @ X_j
        ob = const.tile([128, 512], bf, tag="ob", name="ob")
        for h in range(2):
            X = const.tile([128, 2048], bf, tag=f"X{h}", name=f"X{h}")
            X_v = X[:].rearrange("p (j c r l) -> p j c r l", j=4, c=4, r=16)
            wx_v = wx[h][:].rearrange("p (j r l) -> p j r l", j=4, r=16)
            for pr in range(2):
                j0 = 2 * pr
                wx_b = wx_v[:, j0:j0 + 2, :, :].unsqueeze(2)
                wx_b = wx_b.broadcast_to([128, 2, 4, 16, 8])
                caw_b = caw_v[:, 4 * h + j0:4 * h + j0 + 2, :, :].unsqueeze(3)
                caw_b = caw_b.broadcast_to([128, 2, 4, 16, 8])
                nc.vector.tensor_tensor(out=X_v[:, j0:j0 + 2], in0=wx_b,
                                        in1=caw_b, op=mult)
                for j in (j0, j0 + 1):
                    nc.tensor.matmul(
                        po[:], lhsT=wy[h][:, 128 * j:128 * j + 128],
                        rhs=X[:, 512 * j:512 * j + 512],
                        start=(h == 0 and j == 0), stop=(h == 1 and j == 3))
        nc.vector.tensor_copy(out=ob[:, 0:256], in_=po[:, 0:256])
        nc.scalar.activation(out=ob[:, 256:512], in_=po[:, 256:512], func=CPY)
        nc.sync.dma_start(out=out_d, in_=ob[:])

    nc.compile()
    return nc


def _split3(v, bf):
    """Split f64 array v into 3 bf16 pieces summing to ~24-bit accuracy."""
    s1 = v.astype(bf)
    s2 = (v - s1.astype(np.float64)).astype(bf)
    s3 = (v - s1.astype(np.float64) - s2.astype(np.float64)).astype(bf)
    return s1, s2, s3


def _host_prep(positions, colors, opacities, scales, qvec, tvec):
    """Build the 8 per-core input maps (all projection/coef math on host)."""
    import ml_dtypes
    bf = ml_dtypes.bfloat16

    positions = np.asarray(positions, dtype=np.float64)
    colors = np.asarray(colors, dtype=np.float64)
    opacities = np.asarray(opacities, dtype=np.float64)
    scales = np.asarray(scales, dtype=np.float64)
    qvec = np.asarray(qvec, dtype=np.float64)
    tvec = np.asarray(tvec, dtype=np.float64)

    gneg = -0.5 / (scales[:, 0] ** 2)            # [N]
    cav = np.concatenate([colors * opacities, opacities], axis=1)  # [N,4]

    # 4-chunk block-diagonal basis [128, 512] (blocks in rows 0:64)
    q = np.arange(128.0) - 64.0
    p2 = q * q
    p2h = p2.astype(bf)
    p2l = (p2 - p2h.astype(np.float64)).astype(bf)
    bas = np.zeros((128, 512), bf)
    for j in range(4):
        r0, c0 = 16 * j, 128 * j
        for r in (0, 2, 4):
            bas[r0 + r, c0:c0 + 128] = p2h
            bas[r0 + r + 1, c0:c0 + 128] = p2l
        for r in (6, 7, 8):
            bas[r0 + r, c0:c0 + 128] = q.astype(bf)
        for r in (9, 10, 11):
            bas[r0 + r, c0:c0 + 128] = 1.0

    def coefT_half(a_c, g_c):
        """[128, 128] bf16: rows 16jl+r (rows 0:64), cols m, 4 chunks."""
        out = np.zeros((128, 128), bf)
        gg = g_c.reshape(4, 128)
        b = (-2.0 * g_c * a_c).reshape(4, 128)
        cc = (g_c * a_c * a_c).reshape(4, 128)
        for j in range(4):
            a1, a2, a3 = _split3(gg[j], bf)
            for r, v in ((0, a1), (1, a1), (2, a2), (3, a2), (4, a3), (5, a3)):
                out[16 * j + r, :] = v
            b1, b2, b3 = _split3(b[j], bf)
            for r, v in ((6, b1), (7, b2), (8, b3)):
                out[16 * j + r, :] = v
            c1, c2, c3 = _split3(cc[j], bf)
            for r, v in ((9, c1), (10, c2), (11, c3)):
                out[16 * j + r, :] = v
        return out

    in_maps = []
    for p in range(NPOSE):
        R = _quat2mat(qvec[p])
        t = tvec[p]
        A = np.zeros((3, 4))
        A[0, :3] = FX * R[0]
        A[0, 3] = FX * t[0]
        A[1, :3] = FY * R[1]
        A[1, 3] = FY * t[1]
        A[2, :3] = R[2]
        A[2, 3] = t[2]
        cam = positions @ A[:, :3].T + A[:, 3]   # [N,3]
        ax = cam[:, 0] / cam[:, 2]               # centered (cx=64 -> q=px-64)
        ay = cam[:, 1] / cam[:, 2]

        for s in range(NSHARD):
            sl = slice(s * NG, s * NG + NG)
            axs, ays, gs = ax[sl], ay[sl], gneg[sl]
            d1 = np.zeros((128, 640), bf)
            d1[:, 0:512] = bas
            d1[:, 512:640] = coefT_half(axs[0:512], gs[0:512])
            d2 = np.zeros((128, 256), bf)
            d2[:, 0:128] = coefT_half(ays[0:512], gs[0:512])
            d2[:, 128:256] = coefT_half(axs[512:1024], gs[512:1024])
            d3 = np.zeros((128, 384), bf)
            d3[:, 0:128] = coefT_half(ays[512:1024], gs[512:1024])
            # caw8[m, (j,c,l)]: l = 8 replicas
            cv = cav[sl].reshape(NCHUNK, 128, 4).astype(bf)   # [j, m, c]
            d3[:, 128:384] = np.broadcast_to(
                cv.transpose(1, 0, 2)[:, :, :, None],
                (128, NCHUNK, 4, 8)).reshape(128, 256)
            in_maps.append({"d1": d1, "d2": d2, "d3": d3})
    return in_maps


def _assemble(slabs):
    """slabs: 8 x [128, 512] partials -> [NPOSE*16, 3, 32, 32] output."""
    out = []
    for p in range(NPOSE):
        acc = np.zeros((128, 512), np.float64)
        for sl in slabs[4 * p:4 * p + 4]:
            acc += sl.astype(np.float64)
        den = acc[:, 384:512] + 1e-8             # [py, px]
        img = np.empty((H, W, 3), np.float64)
        for c in range(3):
            img[:, :, c] = acc[:, 128 * c:128 * c + 128] / den
        tiles = img.reshape(H * W, 3).reshape(16, 1024, 3)
        tiles = tiles.transpose(0, 2, 1).reshape(16, 3, 32, 32)
        out.append(tiles)
    return np.concatenate(out, axis=0).astype(F32)


def kernel(positions, colors, opacities, scales, qvec, tvec, _trace=False):
    from concourse.bass_utils import run_bass_kernel_spmd

    if "nc" not in _CACHE:
        _CACHE["nc"] = _build_program()
    nc = _CACHE["nc"]

    in_maps = _host_prep(positions, colors, opacities, scales, qvec, tvec)
    res = run_bass_kernel_spmd(nc, in_maps, core_ids=list(range(8)),
                               trace=_trace)
    slabs = [np.asarray(res.results[c]["out"]) for c in range(8)]
    out = _assemble(slabs)
    if _trace:
        _CACHE["last_result"] = res
    return out
